# revision 18
# baseline (speedup 1.0000x reference)
"""Trainium2 Bass kernel for nn_BoundaryUnit (sparse_attention, memory-bound).

8-core SPMD strategy:
  - f_m [B,N,N,D] sharded over the first N axis (i): core c owns i in
    [16c,16c+16).  Host sums the per-core partial outputs (psum over
    shards; reduction is over the sharded dim).
  - Rotation trick: all n-indexed inputs are rotated by -16c so every
    core runs the identical program with i-rows at positions 0..15;
    host un-rotates the outputs.
  - silu trick: sigmoid(m*s)*m == silu(m*s)/s -> one DVE multiply (x s)
    + one ACT Silu pass per element; the /s is folded into a single
    per-batch PSUM finalize (x 8/s; host divides the summed result by 8).
  - A_b-weighted i-reduction on the PE: psum += diag(A^T[:,i]) @ u_i,
    bf16 operands, fp32 accumulate.  diag built on ACT (Copy w/
    per-partition scale) - Copy lives in every ACT table set, so the
    Exp (softmax) -> Silu switch happens exactly once.
  - Small attention path in bf16 matmuls (fp32 PSUM, fp32 softmax),
    moving operands b-stacked to amortize LDWEIGHTS.
"""

import sys

for _p in ("/opt/trn_rl_repo",):
    if _p not in sys.path:
        sys.path.insert(0, _p)

import numpy as np
import ml_dtypes

import concourse.bass as bass
import concourse.mybir as mybir
from concourse.bass_utils import run_bass_kernel_spmd
from concourse.tile import TileContext

B, N, L, D = 4, 128, 20, 512
NCORES = 8
NI = N // NCORES          # i-rows per core
KC = D // 128             # 128-row chunks of D
GI = 4                    # i's per DMA/elementwise group
NG = NI // GI             # groups per (b, core)
SCALE = float(1.0 / np.sqrt(D))

F32 = mybir.dt.float32
F32R = mybir.dt.float32r
BF16 = mybir.dt.bfloat16
AF = mybir.ActivationFunctionType
ALU = mybir.AluOpType
AX = mybir.AxisListType

CFG = dict(
    bcast_dma=True,        # broadcast [1,X] DRAM rows across 128 partitions
    gate_attn_group=1,     # attn-softmax exps wait for this silu group
    gate_A_group=5,        # A-softmax exps wait for this silu group
    dma_cast=True,         # cast f_m to bf16 in the DMA (SWDGE)
    dma_accum_out=True,    # accumulate small-path into out via DMA
)

MAX_WAITS = 1  # this walrus build allows 1 sync-wait per instruction


def _split_excess_waits(nc):
    for fn in nc.m.functions:
        for blk in fn.blocks:
            out = []
            for inst in blk.instructions:
                si = inst.sync_info
                if si is not None and si.on_wait is not None and len(si.on_wait) > MAX_WAITS:
                    waits = list(si.on_wait)
                    excess, keep = waits[:-MAX_WAITS], waits[-MAX_WAITS:]
                    for ci in range(0, len(excess), MAX_WAITS):
                        out.append(mybir.InstNoOp(
                            name=f"{inst.name}-wsplit-{ci}",
                            engine=inst.engine,
                            sync_info=mybir.SyncInfo(
                                on_wait=list(excess[ci:ci + MAX_WAITS]), on_update=[]),
                        ))
                    si.on_wait = keep
                out.append(inst)
            blk.instructions = out


def build_nc():
    nc = bass.Bass("TRN2", target_bir_lowering=False, debug=False)

    fm = nc.dram_tensor("fm", [B, NI, N, D], F32, kind="ExternalInput").ap()
    fb = nc.dram_tensor("fb", [B, N, D], F32, kind="ExternalInput").ap()
    fbc = nc.dram_tensor("fbc", [B, N, D], BF16, kind="ExternalInput").ap()
    fbT = nc.dram_tensor("fbT", [B, D, N], BF16, kind="ExternalInput").ap()
    wqT = nc.dram_tensor("wqT", [D, D], BF16, kind="ExternalInput").ap()
    wkT = nc.dram_tensor("wkT", [D, D], BF16, kind="ExternalInput").ap()
    fw = nc.dram_tensor("fw", [B, L, D], BF16, kind="ExternalInput").ap()
    fwT = nc.dram_tensor("fwT", [B, D, L], BF16, kind="ExternalInput").ap()
    bq_c = nc.dram_tensor("bq_c", [N, KC], F32, kind="ExternalInput").ap()
    bk_c = nc.dram_tensor("bk_c", [N, KC], F32, kind="ExternalInput").ap()
    fs_c = nc.dram_tensor("fs_c", [N, B * KC], F32, kind="ExternalInput").ap()
    eyeb_d = nc.dram_tensor("eyeb", [N, N], BF16, kind="ExternalInput").ap()
    cb_d = nc.dram_tensor("cb", [N, 2], F32, kind="ExternalInput").ap()
    out = nc.dram_tensor("out", [B, N, D], F32, kind="ExternalOutput").ap()
    fs_rep_d = nc.dram_tensor("fs_rep", [N, B * D], BF16, kind="ExternalInput").ap()
    iv8_rep_d = nc.dram_tensor("iv8_rep", [N, B * D], F32, kind="ExternalInput").ap()

    with TileContext(nc) as tc:
        with (
            tc.tile_pool(name="const", bufs=1) as cpool,
            tc.tile_pool(name="small", bufs=1) as spool,
            tc.tile_pool(name="mg", bufs=4) as mgpool,
            tc.tile_pool(name="t0", bufs=6) as t0pool,
            tc.tile_pool(name="u", bufs=16) as upool,
            tc.tile_pool(name="dg", bufs=3) as dgpool,
            tc.tile_pool(name="fin", bufs=2) as fpool,
            tc.tile_pool(name="ps", bufs=6, space="PSUM") as pspool,
            tc.tile_pool(name="pmom", bufs=2, space="PSUM") as pmpool,
        ):
            def load(pool, src, shape, dtype=F32, tag="t"):
                t = pool.tile(shape, dtype, tag=tag, name=tag)
                nc.sync.dma_start(t[:], src)
                return t

            # ---- constants (few big DMAs via 3D APs) ----
            fsr = cpool.tile([N, B * D], BF16, tag="fsr", name="fsr")
            nc.scalar.dma_start(fsr[:], fs_rep_d[:])
            wq_all = cpool.tile([128, KC * D], BF16, tag="wq", name="wq")
            nc.scalar.dma_start(wq_all[:].rearrange("p (c d) -> p c d", c=KC),
                              wqT[:].rearrange("(c p) d -> p c d", c=KC))
            wq_t = [wq_all[:, kc * D:(kc + 1) * D] for kc in range(KC)]
            wk_all = cpool.tile([128, KC * D], BF16, tag="wk", name="wk")
            nc.sync.dma_start(wk_all[:].rearrange("p (c d) -> p c d", c=KC),
                              wkT[:].rearrange("(c p) d -> p c d", c=KC))
            wk_t = [wk_all[:, kc * D:(kc + 1) * D] for kc in range(KC)]
            # b-stacked moving operands: fbT_all[kc][:, b*128:(b+1)*128] = fbT[b, kc-chunk]
            fbT_big = cpool.tile([128, KC * B * N], BF16, tag="fbTa", name="fbTa")
            for kc in range(KC):
                nc.scalar.dma_start(
                    fbT_big[:, kc * B * N:(kc + 1) * B * N].rearrange("p (b n) -> p b n", b=B),
                    fbT[:, kc * 128:(kc + 1) * 128, :].rearrange("b p n -> p b n"))
            fbT_all = [fbT_big[:, kc * B * N:(kc + 1) * B * N] for kc in range(KC)]
            fwT_big = cpool.tile([128, KC * B * L], BF16, tag="fwTa", name="fwTa")
            for kc in range(KC):
                nc.sync.dma_start(
                    fwT_big[:, kc * B * L:(kc + 1) * B * L].rearrange("p (b l) -> p b l", b=B),
                    fwT[:, kc * 128:(kc + 1) * 128, :].rearrange("b p l -> p b l"))
            fwT_all = [fwT_big[:, kc * B * L:(kc + 1) * B * L] for kc in range(KC)]
            fb_big = cpool.tile([N, B * D], F32, tag="fbb", name="fbb")
            nc.sync.dma_start(fb_big[:].rearrange("p (b d) -> p b d", b=B),
                              fb[:].rearrange("b n d -> n b d"))
            fb_t = [fb_big[:, b * D:(b + 1) * D] for b in range(B)]
            fbc_big = cpool.tile([N, B * D], BF16, tag="fbc", name="fbc")
            nc.sync.dma_start(fbc_big[:].rearrange("p (b d) -> p b d", b=B),
                              fbc[:].rearrange("b n d -> n b d"))
            fbc_t = [fbc_big[:, b * D:(b + 1) * D] for b in range(B)]
            fw_big = cpool.tile([L, B * D], BF16, tag="fwb", name="fwb")
            nc.sync.dma_start(fw_big[:].rearrange("p (b d) -> p b d", b=B),
                              fw[:].rearrange("b l d -> l b d"))
            fw_t = [fw_big[:, b * D:(b + 1) * D] for b in range(B)]
            eyeb = load(cpool, eyeb_d[:], [N, N], BF16, tag="eyeb")
            cb = load(cpool, cb_d[:], [N, 2], F32, tag="cb")
            bq_t = load(cpool, bq_c[:], [N, KC], F32, tag="bq")
            bk_t = load(cpool, bk_c[:], [N, KC], F32, tag="bk")
            fs_t = load(cpool, fs_c[:], [N, B * KC], F32, tag="fs")
            iv8 = cpool.tile([N, B * D], F32, tag="iv8", name="iv8")
            nc.sync.dma_start(iv8[:], iv8_rep_d[:])

            # ---- moment elementwise pipeline (consts-only deps) ----
            u_tiles = {}
            gate_attn = spool.tile([N, 1], F32, tag="g_attn", name="g_attn")
            gate_A = spool.tile([N, 1], F32, tag="g_A", name="g_A")
            gidx = 0
            for b in range(B):
                for g in range(NG):
                    cast = CFG["dma_cast"] and (gidx % 2 == 0)
                    mg = mgpool.tile([N, GI * D], BF16 if cast else F32,
                                     tag="mgc" if cast else "mgf", name="mg")
                    dma_eng = nc.gpsimd if cast else nc.sync
                    dma_eng.dma_start(
                        mg[:].rearrange("p (i d) -> p i d", i=GI),
                        fm[b, g * GI:(g + 1) * GI, :, :].rearrange("i j d -> j i d"))
                    t0 = t0pool.tile([N, GI * D], BF16, tag="t0", name="t0")
                    nc.vector.tensor_mul(
                        t0[:].rearrange("p (i d) -> p i d", i=GI),
                        mg[:].rearrange("p (i d) -> p i d", i=GI),
                        fsr[:, b * D:(b + 1) * D].rearrange("p (i d) -> p i d", i=1).broadcast_to([N, GI, D]))
                    ut = upool.tile([N, GI * D], BF16, tag="u", name="ut")
                    nc.scalar.activation(ut[:], t0[:], AF.Silu)
                    u_tiles[(b, g)] = ut
                    if gidx == CFG["gate_attn_group"]:
                        nc.vector.scalar_tensor_tensor(
                            gate_attn[:], ut[:, 0:1], 0.0, cb[:, 0:1],
                            op0=ALU.mult, op1=ALU.add)
                    if gidx == CFG["gate_A_group"]:
                        nc.vector.scalar_tensor_tensor(
                            gate_A[:], ut[:, 0:1], 0.0, cb[:, 1:2],
                            op0=ALU.mult, op1=ALU.add)
                    gidx += 1

            # ---- small path (highest scheduler priority) ----
            hp = tc.high_priority(offset=1000000)
            hp.__enter__()
            qT_sb, kT_sb, fbqT_sb, AT_sb, small_t = {}, {}, {}, {}, {}
            for mc in range(KC):
                p_qT = pspool.tile([128, B * N], F32, tag="ps")
                for kc in range(KC):
                    nc.tensor.matmul(p_qT[:], wq_t[kc][:, mc * 128:(mc + 1) * 128],
                                     fbT_all[kc][:], start=(kc == 0), stop=(kc == KC - 1))
                tq = spool.tile([128, B * N], BF16, tag=f"qT{mc}")
                nc.scalar.activation(tq[:], p_qT[:], AF.Identity, bias=bq_t[:, mc:mc + 1])
                for b in range(B):
                    qT_sb[(b, mc)] = tq[:, b * N:(b + 1) * N]
            for mc in range(KC):
                p_kT = pspool.tile([128, B * L], F32, tag="ps")
                for kc in range(KC):
                    nc.tensor.matmul(p_kT[:], wk_t[kc][:, mc * 128:(mc + 1) * 128],
                                     fwT_all[kc][:], start=(kc == 0), stop=(kc == KC - 1))
                tk = spool.tile([128, B * L], BF16, tag=f"kT{mc}")
                nc.scalar.activation(tk[:], p_kT[:], AF.Identity, bias=bk_t[:, mc:mc + 1])
                for b in range(B):
                    kT_sb[(b, mc)] = tk[:, b * L:(b + 1) * L]

            for b in range(B):
                p_S = pspool.tile([N, L], F32, tag="ps")
                for kc in range(KC):
                    nc.tensor.matmul(p_S[:], qT_sb[(b, kc)], kT_sb[(b, kc)],
                                     start=(kc == 0), stop=(kc == KC - 1))
                a_e = spool.tile([N, L], F32, tag="a_e")
                ssum = spool.tile([N, 1], F32, tag="ssum")
                nc.scalar.activation(a_e[:], p_S[:], AF.Exp, bias=gate_attn[:], scale=SCALE,
                                     accum_out=ssum[:])
                rcp = spool.tile([N, 1], F32, tag="rcp")
                nc.vector.reciprocal(rcp[:], ssum[:])
                a_n = spool.tile([N, L], BF16, tag="a_n")
                nc.vector.tensor_scalar(a_n[:], a_e[:], rcp[:], None, ALU.mult)
                p_aT = pspool.tile([L, N], BF16, tag="ps")
                nc.tensor.transpose(p_aT[:], a_n[:], eyeb[:])
                aT = spool.tile([L, N], BF16, tag="aT")
                nc.vector.tensor_copy(aT[:], p_aT[:])
                for mc in range(KC):
                    p_fq = pspool.tile([128, N], F32, tag="ps")
                    nc.tensor.matmul(p_fq[:], fw_t[b][:, mc * 128:(mc + 1) * 128], aT[:],
                                     start=True, stop=True)
                    t = spool.tile([128, N], BF16, tag=f"fbqT{b}_{mc}")
                    nc.vector.scalar_tensor_tensor(
                        t[:], p_fq[:], fs_t[:, b * KC + mc:b * KC + mc + 1],
                        fbT_all[mc][:, b * N:(b + 1) * N], op0=ALU.add, op1=ALU.mult)
                    fbqT_sb[(b, mc)] = t
                p_S2 = pspool.tile([N, N], F32, tag="ps")
                for kc in range(KC):
                    nc.tensor.matmul(p_S2[:], fbqT_sb[(b, kc)][:], fbqT_sb[(b, kc)][:],
                                     start=(kc == 0), stop=(kc == KC - 1))
                A_e = spool.tile([N, N], F32, tag="A_e")
                ssum2 = spool.tile([N, 1], F32, tag="ssum2")
                nc.scalar.activation(A_e[:], p_S2[:], AF.Exp, bias=gate_A[:], scale=SCALE,
                                     accum_out=ssum2[:])
                rcp2 = spool.tile([N, 1], F32, tag="rcp2")
                nc.vector.reciprocal(rcp2[:], ssum2[:])
                A_n = spool.tile([N, N], BF16, tag="A_n")
                nc.vector.tensor_scalar(A_n[:], A_e[:], rcp2[:], None, ALU.mult)
                p_AT = pspool.tile([N, N], BF16, tag="ps")
                nc.tensor.transpose(p_AT[:], A_n[:], eyeb[:])
                t_AT = spool.tile([N, N], BF16, tag=f"AT{b}")
                nc.vector.tensor_copy(t_AT[:], p_AT[:])
                AT_sb[b] = t_AT
                p_fbb = pspool.tile([N, D], F32, tag="ps")
                nc.tensor.matmul(p_fbb[:], t_AT[:], fbc_t[b], start=True, stop=True)
                st = spool.tile([N, D], F32, tag=f"small{b}")
                nc.vector.tensor_add(st[:], p_fbb[:], fb_t[b])
                small_t[b] = st

            # ---- moment path ----
            hp.__exit__(None, None, None)
            for b in range(B):
                p_mom = pmpool.tile([N, D], F32, tag="mom")
                for g in range(NG):
                    dgc = dgpool.tile([N, GI * N], BF16, tag="dg", name="dgc")
                    nc.vector.tensor_mul(
                        dgc[:].rearrange("p (i n) -> p i n", i=GI),
                        eyeb[:].rearrange("p (i n) -> p i n", i=1).broadcast_to([N, GI, N]),
                        AT_sb[b][:, g * GI:(g + 1) * GI].rearrange("p (i n) -> p i n", n=1).broadcast_to([N, GI, N]))
                    ut = u_tiles[(b, g)]
                    for il in range(GI):
                        i16 = g * GI + il
                        nc.tensor.matmul(p_mom[:], dgc[:, il * N:(il + 1) * N],
                                         ut[:, il * D:(il + 1) * D],
                                         start=(i16 == 0), stop=(i16 == NI - 1))
                mo = fpool.tile([N, D], F32, tag="mo")
                nc.vector.tensor_mul(mo[:], p_mom[:], iv8[:, b * D:(b + 1) * D])
                if CFG["dma_accum_out"]:
                    nc.gpsimd.dma_start(out[b], mo[:])
                    nc.gpsimd.dma_start(out[b], small_t[b][:], accum_op=ALU.add)
                else:
                    ot = fpool.tile([N, D], F32, tag="ot")
                    nc.vector.tensor_add(ot[:], mo[:], small_t[b][:])
                    nc.sync.dma_start(out[b], ot[:])

    _split_excess_waits(nc)
    return nc


_CACHE = {}


def _get_nc():
    if "nc" not in _CACHE:
        _CACHE["nc"] = build_nc()
    return _CACHE["nc"]


def _prep_in_maps(f_b, f_w, f_s, f_m, Wq, bq, Wk, bk):
    f_b = np.ascontiguousarray(f_b, np.float32)
    f_w = np.ascontiguousarray(f_w, np.float32)
    f_s = np.ascontiguousarray(f_s, np.float32)
    f_m = np.ascontiguousarray(f_m, np.float32)
    bf = ml_dtypes.bfloat16

    wqT = np.ascontiguousarray(np.asarray(Wq, np.float32).T.astype(bf))
    wkT = np.ascontiguousarray(np.asarray(Wk, np.float32).T.astype(bf))
    fw_b = f_w.astype(bf)
    fwT = np.ascontiguousarray(f_w.transpose(0, 2, 1).astype(bf))
    bq_c = np.ascontiguousarray(np.asarray(bq, np.float32).reshape(KC, 128).T)
    bk_c = np.ascontiguousarray(np.asarray(bk, np.float32).reshape(KC, 128).T)
    fs_cm = np.ascontiguousarray(
        f_s.reshape(B, KC, 128).transpose(2, 0, 1).reshape(128, B * KC))
    inv8 = (8.0 / f_s.astype(np.float64)).astype(np.float32)
    eyeb = np.eye(N, dtype=bf)

    common = {
        "wqT": wqT, "wkT": wkT, "fw": fw_b, "fwT": fwT,
        "bq_c": bq_c, "bk_c": bk_c, "fs_c": fs_cm, "eyeb": eyeb,
        "cb": np.ascontiguousarray(np.broadcast_to(np.array([[0.0, -46.0]], np.float32), (N, 2))),
    }
    common["fs_rep"] = np.ascontiguousarray(
        np.broadcast_to(f_s.reshape(1, B * D).astype(bf), (N, B * D)))
    common["iv8_rep"] = np.ascontiguousarray(
        np.broadcast_to(inv8.reshape(1, B * D), (N, B * D)))

    in_maps = []
    for c in range(NCORES):
        r = -NI * c
        fb_c = np.ascontiguousarray(np.roll(f_b, r, axis=1))
        fm_c = np.ascontiguousarray(np.roll(f_m, r, axis=2)[:, NI * c:NI * (c + 1)])
        m = dict(common)
        m["fm"] = fm_c
        m["fb"] = fb_c
        m["fbT"] = np.ascontiguousarray(fb_c.transpose(0, 2, 1).astype(bf))
        m["fbc"] = np.ascontiguousarray(fb_c.astype(bf))
        in_maps.append(m)
    return in_maps


def _run(in_maps, **kwargs):
    nc = _get_nc()
    return run_bass_kernel_spmd(nc, in_maps, core_ids=list(range(NCORES)), **kwargs)


def kernel(f_b, f_w, f_s, f_m, Wq, bq, Wk, bk, _run_kwargs=None, _return_raw=False):
    in_maps = _prep_in_maps(f_b, f_w, f_s, f_m, Wq, bq, Wk, bk)
    res = _run(in_maps, **(_run_kwargs or {}))
    total = np.zeros((B, N, D), np.float32)
    for c in range(NCORES):
        total += np.roll(res.results[c]["out"], NI * c, axis=1)
    total *= np.float32(0.125)
    if _return_raw:
        return total, res
    return total
